# revision 12
# baseline (speedup 1.0000x reference)
"""LocallyConnected2d Bass kernel for 8 Trainium2 NeuronCores.

Problem (hardcoded): x[16,32,64,64] f32, weight[64,64,32,32,3,3] f32,
bias[32,64,64] f32 -> out[16,32,64,64] f32.  stride=1, pad=1, dil=1.

Sharding: outH split across 8 cores (8 rows each).  Per core, per output
row h: 64 w-positions x 3 kernel-rows of matmuls [K<=97,M=32]x[K,N=16]
accumulated in PSUM.  K = (kernel-col j)*32 + inC c, with a 97th "ones"
row carrying the bias.  Weights/x are cast to bf16 on host (f32 PSUM
accumulation); all SBUF layouts are precomputed host-side so the device
only does plain contiguous DMAs.

w-positions are processed in quads: position w = q*4+g is computed by a
matmul col-tiled to column group g (tile_position=(0,32g)), so the four
LDWEIGHTS+MATMUL streams of a quad run concurrently in the PE array.
PSUM tile is [128 = 4w x 32o, 16 quads x 16b] per output row.
"""

import numpy as np
import ml_dtypes

B, C, H, W = 16, 32, 64, 64
OC = 32
KH = KW = 3
NCORES = 8
RPC = H // NCORES  # rows per core = 8
WCH = 16  # w-positions per weight chunk
NQ = 4  # quad size (PE col groups)

BF16 = ml_dtypes.bfloat16

# x tile chunking by padded row hh: chunk -> (hh0, hh1)
XCHUNKS = [(0, 3), (3, 7), (7, 10)]

_cache = {}


def _build_nc():
    import concourse.bass as bass
    import concourse.tile as tile
    from concourse import bacc, mybir

    nc = bacc.Bacc(
        "TRN2", target_bir_lowering=False, debug=False, num_devices=NCORES
    )
    f32 = mybir.dt.float32
    f16 = mybir.dt.float16
    bf16 = mybir.dt.bfloat16

    # xs: [97, 10, 64*16] bf16.  Partition j*32+c holds x[c, hh, w+j, b]
    # (hh = local padded row 0..9, w = out col, b = batch); row 96 = 1.0.
    xs = nc.dram_tensor("xs", (97, 10, W * B), bf16, kind="ExternalInput")
    # wt: [8, 4, 97, 16*3*32] bf16; [h, chunk, j*32+c, (wl*3+ik)*32+o];
    # row 96 holds bias at ik==2 slots, zeros elsewhere.
    wt = nc.dram_tensor(
        "wt", (RPC, W // WCH, 97, WCH * KH * OC), bf16, kind="ExternalInput"
    )
    # out: [8, 128, 16*16] f16 = out[h, g*32+o, q*16+b] with w = q*4+g
    out = nc.dram_tensor(
        "out", (RPC, 4 * OC, (W // NQ) * B), f16, kind="ExternalOutput"
    )

    with tile.TileContext(nc) as tc:
        with (
            tc.tile_pool(name="xpool", bufs=1) as xpool,
            tc.tile_pool(name="wpool", bufs=6) as wpool,
            tc.tile_pool(name="opool", bufs=4) as opool,
            tc.tile_pool(name="psum", bufs=3, space="PSUM") as ppool,
        ):
            # x row-chunks as separate tiles for fine-grained DMA deps.
            # x rides the scalar HWDGE ring so weight DMAs (sync ring)
            # stream in parallel from t=0.
            xtiles = []
            for ci, (h0, h1) in enumerate(XCHUNKS):
                t = xpool.tile([97, h1 - h0, W * B], bf16, tag=f"x{ci}")
                nc.scalar.dma_start(t[:], xs[:, h0:h1])
                xtiles.append(t)

            def xslice(hh, w, k):
                for (h0, h1), t in zip(XCHUNKS, xtiles):
                    if h0 <= hh < h1:
                        return t[0:k, hh - h0, w * B : (w + 1) * B]
                raise AssertionError

            for h in range(RPC):
                wtiles = []
                for ch in range(W // WCH):
                    wti = wpool.tile([97, WCH * KH * OC], bf16)
                    nc.sync.dma_start(wti[:], wt[h, ch])
                    wtiles.append(wti)

                pt = ppool.tile([4 * OC, (W // NQ) * B], f32)
                for q in range(W // NQ):
                    for g in range(NQ):
                        w = q * NQ + g
                        wti = wtiles[w // WCH]
                        wl = w % WCH
                        for ik in range(KH):
                            k = 97 if ik == 2 else 96
                            woff = (wl * 3 + ik) * 32
                            nc.tensor.matmul(
                                pt[32 * g : 32 * (g + 1), q * B : (q + 1) * B],
                                wti[0:k, woff : woff + 32],
                                xslice(h + ik, w, k),
                                start=(ik == 0),
                                stop=(ik == 2),
                                tile_position=(0, 32 * g),
                            )
                half = (W // NQ) * B // 2
                for piece in range(2):
                    ot = opool.tile([4 * OC, half], f16)
                    sl = slice(piece * half, (piece + 1) * half)
                    nc.vector.tensor_copy(ot[:], pt[:, sl])
                    nc.scalar.dma_start(out[h, :, sl], ot[:])
    nc.compile()
    return nc


def _prep_inputs(x, weight, bias):
    """Host-side shard + layout prep.  Returns list of 8 per-core dicts."""
    # padded x, transposed to [c, hh, wp, b]
    xp = np.zeros((C, H + 2, W + 2, B), dtype=BF16)
    xp[:, 1 : H + 1, 1 : W + 1, :] = np.ascontiguousarray(
        x.transpose(1, 2, 3, 0)
    ).astype(BF16)

    # weight -> [h, j, c, w, ik, o]
    wtr = np.ascontiguousarray(weight.transpose(0, 5, 3, 1, 4, 2)).astype(BF16)
    wtr = wtr.reshape(H, 96, W, KH, OC)
    btr = bias.transpose(1, 2, 0).astype(BF16)  # [h, w, o]

    in_maps = []
    for i in range(NCORES):
        h0 = i * RPC
        # xs replication: [97, 10, 64, 16]
        rep = np.zeros((97, RPC + 2, W, B), dtype=BF16)
        slab = xp[:, h0 : h0 + RPC + 2, :, :]  # [32, 10, 66, 16]
        for j in range(KW):
            rep[j * 32 : (j + 1) * 32] = slab[:, :, j : j + W, :]
        rep[96] = 1.0

        wcore = np.zeros((RPC, 97, W, KH, OC), dtype=BF16)
        wcore[:, 0:96] = wtr[h0 : h0 + RPC]
        wcore[:, 96, :, 2, :] = btr[h0 : h0 + RPC]  # bias via ones-row, ik==2
        # -> [h, chunk, 97, WCH*3*32]
        wcore = wcore.reshape(RPC, 97, W // WCH, WCH * KH * OC).transpose(
            0, 2, 1, 3
        )

        in_maps.append(
            {
                "xs": np.ascontiguousarray(rep.reshape(97, RPC + 2, W * B)),
                "wt": np.ascontiguousarray(wcore),
            }
        )
    return in_maps


def _run(in_maps, trace=False, tmpdir=None):
    from concourse.bass_utils import run_bass_kernel_spmd

    if "nc" not in _cache:
        _cache["nc"] = _build_nc()
    return run_bass_kernel_spmd(
        _cache["nc"], in_maps, list(range(NCORES)), trace=trace, tmpdir=tmpdir
    )


def _assemble(results):
    out = np.empty((B, OC, H, W), dtype=np.float32)
    for i in range(NCORES):
        # res: [h, g*32+o, q*16+b], w = q*4+g
        res = (
            results[i]["out"].astype(np.float32).reshape(RPC, NQ, OC, W // NQ, B)
        )
        # -> out[b, o, h, q*4+g]
        out[:, :, i * RPC : (i + 1) * RPC, :] = res.transpose(
            4, 2, 0, 3, 1
        ).reshape(B, OC, RPC, W)
    return out


def kernel(x, weight, bias):
    x = np.asarray(x)
    weight = np.asarray(weight)
    bias = np.asarray(bias)
    in_maps = _prep_inputs(x, weight, bias)
    results = _run(in_maps).results
    return _assemble(results)


# revision 13
# speedup vs baseline: 1.1077x; 1.1077x over previous
"""LocallyConnected2d Bass kernel for 8 Trainium2 NeuronCores.

Problem (hardcoded): x[16,32,64,64] f32, weight[64,64,32,32,3,3] f32,
bias[32,64,64] f32 -> out[16,32,64,64] f32.  stride=1, pad=1, dil=1.

Sharding: outH split across 8 cores (8 rows each).  Per core, per output
row h: 64 w-positions x 3 kernel-rows of matmuls [K<=97,M=32]x[K,N=16]
accumulated in PSUM.  K = (kernel-col j)*32 + inC c, with a 97th "ones"
row carrying the bias.  Weights/x are cast to bf16 on host (f32 PSUM
accumulation); all SBUF layouts are precomputed host-side so the device
only does plain contiguous DMAs.

w-positions are processed in quads: position w = q*4+g is computed by a
matmul col-tiled to column group g (tile_position=(0,32g)), so the four
LDWEIGHTS+MATMUL streams of a quad run concurrently in the PE array.
PSUM tile is [128 = 4w x 32o, 16 quads x 16b] per output row.
"""

import numpy as np
import ml_dtypes

B, C, H, W = 16, 32, 64, 64
OC = 32
KH = KW = 3
NCORES = 8
RPC = H // NCORES  # rows per core = 8
WCH = 16  # w-positions per weight chunk
NQ = 4  # quad size (PE col groups)

BF16 = ml_dtypes.bfloat16

# x tile chunking by padded row hh: chunk -> (hh0, hh1)
XCHUNKS = [(0, 3), (3, 7), (7, 10)]

_cache = {}


def _build_nc():
    import concourse.bass as bass
    import concourse.tile as tile
    from concourse import bacc, mybir

    nc = bacc.Bacc(
        "TRN2", target_bir_lowering=False, debug=False, num_devices=NCORES
    )
    f32 = mybir.dt.float32
    f16 = mybir.dt.float16
    bf16 = mybir.dt.bfloat16

    # xs: [97, 10, 64*16] bf16.  Partition j*32+c holds x[c, hh, w+j, b]
    # (hh = local padded row 0..9, w = out col, b = batch); row 96 = 1.0.
    xs = nc.dram_tensor("xs", (97, 10, W * B), bf16, kind="ExternalInput")
    # wt: [8, 4, 97, 16*3*32] bf16; [h, chunk, j*32+c, (wl*3+ik)*32+o];
    # row 96 holds bias at ik==2 slots, zeros elsewhere.
    wt = nc.dram_tensor(
        "wt", (RPC, W // WCH, 97, WCH * KH * OC), bf16, kind="ExternalInput"
    )
    # out: [8, 128, 16*16] f16 = out[h, g*32+o, q*16+b] with w = q*4+g
    out = nc.dram_tensor(
        "out", (RPC, 4 * OC, (W // NQ) * B), f16, kind="ExternalOutput"
    )

    with tile.TileContext(nc) as tc:
        with (
            tc.tile_pool(name="xpool", bufs=1) as xpool,
            tc.tile_pool(name="wpool", bufs=6) as wpool,
            tc.tile_pool(name="opool", bufs=4) as opool,
            tc.tile_pool(name="psum", bufs=3, space="PSUM") as ppool,
        ):
            # x row-chunks as separate tiles for fine-grained DMA deps.
            # x rides the scalar HWDGE ring so weight DMAs (sync ring)
            # stream in parallel from t=0.
            xtiles = []
            for ci, (h0, h1) in enumerate(XCHUNKS):
                t = xpool.tile([97, h1 - h0, W * B], bf16, tag=f"x{ci}")
                nc.scalar.dma_start(t[:], xs[:, h0:h1])
                xtiles.append(t)

            def xslice(hh, w, k):
                for (h0, h1), t in zip(XCHUNKS, xtiles):
                    if h0 <= hh < h1:
                        return t[0:k, hh - h0, w * B : (w + 1) * B]
                raise AssertionError

            for h in range(RPC):
                wtiles = []
                for ch in range(W // WCH):
                    wti = wpool.tile([97, WCH * KH * OC], bf16)
                    nc.sync.dma_start(wti[:], wt[h, ch])
                    wtiles.append(wti)

                pt = ppool.tile([4 * OC, (W // NQ) * B], f32)
                for q in range(W // NQ):
                    for g in range(NQ):
                        w = q * NQ + g
                        wti = wtiles[w // WCH]
                        wl = w % WCH
                        for ik in range(KH):
                            k = 97 if ik == 2 else 96
                            woff = (wl * 3 + ik) * 32
                            nc.tensor.matmul(
                                pt[32 * g : 32 * (g + 1), q * B : (q + 1) * B],
                                wti[0:k, woff : woff + 32],
                                xslice(h + ik, w, k),
                                start=(ik == 0),
                                stop=(ik == 2),
                                tile_position=(0, 32 * g),
                            )
                ot = opool.tile([4 * OC, (W // NQ) * B], f16)
                nc.vector.tensor_copy(ot[:], pt[:])
                nc.scalar.dma_start(out[h], ot[:])
    nc.compile()
    return nc


def _prep_inputs(x, weight, bias):
    """Host-side shard + layout prep.  Returns list of 8 per-core dicts."""
    # padded x, transposed to [c, hh, wp, b]
    xp = np.zeros((C, H + 2, W + 2, B), dtype=BF16)
    xp[:, 1 : H + 1, 1 : W + 1, :] = np.ascontiguousarray(
        x.transpose(1, 2, 3, 0)
    ).astype(BF16)

    # weight -> [h, j, c, w, ik, o]
    wtr = np.ascontiguousarray(weight.transpose(0, 5, 3, 1, 4, 2)).astype(BF16)
    wtr = wtr.reshape(H, 96, W, KH, OC)
    btr = bias.transpose(1, 2, 0).astype(BF16)  # [h, w, o]

    in_maps = []
    for i in range(NCORES):
        h0 = i * RPC
        # xs replication: [97, 10, 64, 16]
        rep = np.zeros((97, RPC + 2, W, B), dtype=BF16)
        slab = xp[:, h0 : h0 + RPC + 2, :, :]  # [32, 10, 66, 16]
        for j in range(KW):
            rep[j * 32 : (j + 1) * 32] = slab[:, :, j : j + W, :]
        rep[96] = 1.0

        wcore = np.zeros((RPC, 97, W, KH, OC), dtype=BF16)
        wcore[:, 0:96] = wtr[h0 : h0 + RPC]
        wcore[:, 96, :, 2, :] = btr[h0 : h0 + RPC]  # bias via ones-row, ik==2
        # -> [h, chunk, 97, WCH*3*32]
        wcore = wcore.reshape(RPC, 97, W // WCH, WCH * KH * OC).transpose(
            0, 2, 1, 3
        )

        in_maps.append(
            {
                "xs": np.ascontiguousarray(rep.reshape(97, RPC + 2, W * B)),
                "wt": np.ascontiguousarray(wcore),
            }
        )
    return in_maps


def _run(in_maps, trace=False, tmpdir=None):
    from concourse.bass_utils import run_bass_kernel_spmd

    if "nc" not in _cache:
        _cache["nc"] = _build_nc()
    return run_bass_kernel_spmd(
        _cache["nc"], in_maps, list(range(NCORES)), trace=trace, tmpdir=tmpdir
    )


def _assemble(results):
    out = np.empty((B, OC, H, W), dtype=np.float32)
    for i in range(NCORES):
        # res: [h, g*32+o, q*16+b], w = q*4+g
        res = (
            results[i]["out"].astype(np.float32).reshape(RPC, NQ, OC, W // NQ, B)
        )
        # -> out[b, o, h, q*4+g]
        out[:, :, i * RPC : (i + 1) * RPC, :] = res.transpose(
            4, 2, 0, 3, 1
        ).reshape(B, OC, RPC, W)
    return out


def kernel(x, weight, bias):
    x = np.asarray(x)
    weight = np.asarray(weight)
    bias = np.asarray(bias)
    in_maps = _prep_inputs(x, weight, bias)
    results = _run(in_maps).results
    return _assemble(results)
